# revision 35
# baseline (speedup 1.0000x reference)
"""BiDAF attention-flow kernel for Trainium2 (8 NeuronCores, data-parallel over batch).

Problem shapes (hardcoded): h (64, 2048, 200), u (64, 256, 200), w_* (200,), b_* (1,).
Output g (64, 2048, 800) = concat([h, c2q, h*c2q, h*q2c], -1).

Sharding: batch 64 -> 8 cores x 8 batches. Weights replicated. No collectives.

Per-core per-batch math (T=2048, J=256, D=200):
  s[t,j]  = h.w_h[t] + u.w_u[j] + (h*w_hu).u[t,j]   (+ bias, a uniform shift of s that
            cancels in both softmaxes and is therefore dropped; b_* are zeros anyway)
  a       = softmax_j(s); c2q = a @ u
  m[t]    = max_j s ; beta = softmax_t(m); q2c = beta @ h

The kernel computes S^T = umod @ h^T with j on partitions (lhsT = umodT built once per
batch) so the exp'd scores are already j-partitioned for the c2q matmul -- no per-tile
transpose of the softmax output. h^T comes from PE transposes. The constant shift
invariance lets us skip max-subtraction inside both exps (|s| <= ~12 here, exp is safe
in fp32). uw (u.w_u) enters as the per-partition bias of the exp over S^T; hw (h.w_h)
is computed as N=1 f32 matmuls from h^T and folded into the T-softmax weights
multiplicatively (em = rowmax_exp * exp(hw)).

The big matmuls (S^T, c2q, uw) run in float32r: full-rate PE (1 cyc/row at N>=256) vs
4 cyc/row for fp32.  float32r STORAGE rounds values (~tf32), and walrus requires every
producer feeding an f32r matmul to emit f32r -- so the conversions happen in ACT
copies / exp (legal f32r producers), while the g output tile stays exact fp32
(h passes through bit-exact).  f32r matmuls also require N>=2, so the tiny N=1
matmuls (hw, q2c) run in plain fp32 via bitcast views of the f32r hT (bit-exact read).
"""

import sys

sys.path.insert(0, "/opt/trn_rl_repo")

import numpy as np

import concourse.bass as bass
import concourse.bacc as bacc
import concourse.mybir as mybir
import concourse.tile as tile
from concourse.bass_utils import run_bass_kernel_spmd

B, T, J, D = 64, 2048, 256, 200
N_CORES = 8
BL = B // N_CORES          # batches per core
NT = T // 128              # 16 t-tiles per batch
NG = NT // 4               # 4 groups of 512 tokens
F32 = mybir.dt.float32
F32R = mybir.dt.float32r
AX = mybir.AxisListType
AF = mybir.ActivationFunctionType

_cache = {}


def build_nc(use_f32r=True):
    DT = F32R if use_f32r else F32

    def dview(ap):  # DRAM-side view matching DT
        return ap.bitcast(F32R) if use_f32r else ap

    def fview(ap):  # f32 view of a DT-typed AP (bit-exact)
        return ap.bitcast(F32) if use_f32r else ap

    nc = bacc.Bacc()
    h_d = nc.dram_tensor("h", [BL, T, D], F32, kind="ExternalInput")
    u_d = nc.dram_tensor("u", [BL, J, D], F32, kind="ExternalInput")
    wh_d = nc.dram_tensor("w_h", [D], F32, kind="ExternalInput")
    wu_d = nc.dram_tensor("w_u", [D], F32, kind="ExternalInput")
    whu_d = nc.dram_tensor("w_hu", [D], F32, kind="ExternalInput")
    g_d = nc.dram_tensor("g", [BL, T, 4 * D], F32, kind="ExternalOutput")

    with tile.TileContext(nc) as tc:
        with (
            tc.tile_pool(name="sing", bufs=1) as sing,
            tc.tile_pool(name="gpool", bufs=3) as gpool,
            tc.tile_pool(name="upool", bufs=2) as upool,
            tc.tile_pool(name="lhsu", bufs=2) as lhsu,
            tc.tile_pool(name="htp", bufs=2) as htp,
            tc.tile_pool(name="ptp", bufs=2) as ptp,
            tc.tile_pool(name="smalls", bufs=2) as smalls,
            tc.tile_pool(name="rcp", bufs=3) as rcp,
            tc.tile_pool(name="pp", bufs=1, space="PSUM") as pp,
            tc.tile_pool(name="pmc", bufs=2, space="PSUM") as pmc,
            tc.tile_pool(name="pb", bufs=1, space="PSUM") as pb,
        ):
            # ---- per-core constants ----
            # (memset/affine_select can't emit f32r, and every producer feeding an
            # f32r matmul must be f32r-typed -- so constants come via inline DRAM.)
            ident_d = nc.inline_tensor(np.eye(128, dtype=np.float32), name="ident_d")
            ident = sing.tile([128, 128], DT)       # for f32r transposes (pT, uT)
            nc.sync.dma_start(out=ident, in_=dview(ident_d[:, :]))
            ident_f = sing.tile([128, 128], F32)    # for f32 transposes (h, q2c)
            nc.sync.dma_start(out=ident_f, in_=ident_d[:, :])
            ones_d = nc.inline_tensor(np.ones((128, 1), dtype=np.float32), name="ones_d")
            ones_col = sing.tile([128, 1], F32)
            nc.sync.dma_start(out=ones_col, in_=ones_d[:, :])
            # u2 pad columns 200:256 = [1, 0, 0, ...] (denominator ones column)
            upad_d = nc.inline_tensor(
                np.eye(1, 56, dtype=np.float32)[0], name="upad_d"
            )
            wh_col = sing.tile([128, 2], F32)   # hw matmuls run in plain f32 (N=1)
            wu_col = sing.tile([128, 2], DT)
            whu_col = sing.tile([128, 2], F32)  # tensor_scalar scalar must be f32
            wap = dview(wu_d.rearrange("(p o) -> p o", o=1))
            nc.sync.dma_start(out=wu_col[:, 0:1], in_=wap[0:128])
            nc.sync.dma_start(out=wu_col[0:72, 1:2], in_=wap[128:200])
            for w_dram, col in ((wh_d, wh_col), (whu_d, whu_col)):
                wap2 = w_dram.rearrange("(p o) -> p o", o=1)
                nc.sync.dma_start(out=col[:, 0:1], in_=wap2[0:128])
                nc.sync.dma_start(out=col[0:72, 1:2], in_=wap2[128:200])

            def load_batch(b):
                # loads go on the SWDGE (gpsimd) ring so they never queue behind
                # the output stores on the SP HWDGE ring
                g = gpool.tile([128, NT, 4 * D], F32, tag="g", name=f"g{b}")
                u2 = upool.tile([128, 2, 256], DT, tag="u", name=f"u2{b}")
                nc.gpsimd.dma_start(
                    out=g[:, :, 0:D], in_=h_d[b].rearrange("(n p) d -> p n d", p=128)
                )
                nc.gpsimd.dma_start(
                    out=u2[:, :, 0:D],
                    in_=dview(u_d[b].rearrange("(c p) d -> p c d", p=128)),
                )
                # cols 200:256: zero, with a ones column at 200 (softmax denominator)
                upad_ap = bass.AP(
                    tensor=upad_d[:].tensor,
                    offset=upad_d[:].offset,
                    ap=[[0, 128], [0, 2], [1, 56]],
                )
                nc.gpsimd.dma_start(out=u2[:, :, D : D + 56], in_=dview(upad_ap))
                return g, u2

            pending = load_batch(0)
            for b in range(BL):
                g, u2 = pending
                if b + 1 < BL:
                    pending = load_batch(b + 1)

                # ---- u-side prep: uT, uw row, fold w_hu ----
                lhsU0 = lhsu.tile([128, 256], DT, tag="l0")  # umodT rows d=0:128
                lhsU1 = lhsu.tile([128, 256], DT, tag="l1")  # rows 0:72 = d 128:200
                uT0 = pmc.tile([128, 256], DT, tag="mc")
                for c in range(2):
                    nc.tensor.transpose(
                        uT0[:, 128 * c : 128 * (c + 1)], u2[:, c, 0:128], ident
                    )
                nc.scalar.copy(lhsU0, uT0)
                uT1 = pmc.tile([128, 256], DT, tag="mc")
                for c in range(2):
                    nc.tensor.transpose(
                        uT1[0:72, 128 * c : 128 * (c + 1)], u2[:, c, 128:200], ident
                    )
                nc.scalar.copy(lhsU1[0:72], uT1[0:72])
                # uw[j] = sum_d u[j,d] w_u[d]  (raw uT, before the w_hu fold)
                uwp = pb.tile([1, 272], F32, tag="b1")
                nc.tensor.matmul(
                    uwp[0:1, 0:256], wu_col[:, 0:1], lhsU0, start=True, stop=False
                )
                nc.tensor.matmul(
                    uwp[0:1, 0:256], wu_col[0:72, 1:2], lhsU1[0:72],
                    start=False, stop=True,
                )
                uw_sb = smalls.tile([1, 256], F32, tag="uw")
                nc.scalar.copy(uw_sb, uwp[0:1, 0:256])
                # uw as per-j columns: it becomes the bias of the exp over S^T
                # (partitions = j there), so it never enters the matmul at all.
                # Row -> column via two cross-partition SBUF->SBUF DMAs.
                uwcol = smalls.tile([128, 2], F32, tag="uwcol")
                nc.scalar.dma_start(out=uwcol[:, 0:1], in_=uw_sb[0:1, 0:128])
                nc.scalar.dma_start(out=uwcol[:, 1:2], in_=uw_sb[0:1, 128:256])
                # fold w_hu into uT (in place)
                nc.vector.tensor_scalar_mul(lhsU0, lhsU0, whu_col[:, 0:1])
                nc.vector.tensor_scalar_mul(
                    lhsU1[0:72], lhsU1[0:72], whu_col[0:72, 1:2]
                )

                mcol = smalls.tile([128, NT], F32, tag="mcol")
                em = smalls.tile([128, NT], F32, tag="em")
                ehw = smalls.tile([128, NT], F32, tag="ehw")
                # Per-tile matmul scratch: every N=1 matmul writes its OWN column
                # with start=stop=True, so there are no interleaved accumulation
                # groups sharing a bank (those clobber each other's has_written
                # state).  Columns: 0:16 q2c d<128, 16:32 q2c d>=128, 32:48
                # per-tile em sums, 48:64 hw d<128 partial, 64:80 hw d>=128
                # partial (merged later as exp(a+b)=exp(a)*exp(b)).
                pq = pp.tile([128, 80], F32, tag="q", name="pq")

                for gi in range(NG):
                    # h^T for this 512-token group (f32 transposes of the exact
                    # h block; the ACT psum->sbuf copy does the f32r rounding)
                    hT = htp.tile([128, 2, 512], DT, tag="hT")
                    phT0 = pp.tile([128, 512], F32, tag="hT0")
                    phT1 = pp.tile([128, 512], F32, tag="hT1")
                    for k in range(4):
                        i = 4 * gi + k
                        nc.tensor.transpose(
                            phT0[:, 128 * k : 128 * (k + 1)], g[:, i, 0:128], ident_f
                        )
                        nc.tensor.transpose(
                            phT1[0:72, 128 * k : 128 * (k + 1)], g[:, i, 128:200],
                            ident_f,
                        )
                    nc.scalar.copy(hT[:, 0, :], phT0)
                    nc.scalar.copy(hT[0:72, 1, :], phT1[0:72])

                    pT = []
                    for jc in range(2):
                        pST = pp.tile([128, 512], F32, tag=f"ST{jc}")
                        nc.tensor.matmul(
                            pST, lhsU0[:, 128 * jc : 128 * (jc + 1)], hT[:, 0, :],
                            start=True, stop=False,
                        )
                        nc.tensor.matmul(
                            pST, lhsU1[0:72, 128 * jc : 128 * (jc + 1)], hT[0:72, 1, :],
                            start=False, stop=True,
                        )
                        pT_sb = ptp.tile([128, 512], DT, tag=f"pT{jc}", name=f"pT{jc}")
                        # exp(s0 + uw[j]): uw rides as the per-partition bias
                        nc.scalar.activation(
                            pT_sb, pST, AF.Exp, bias=uwcol[:, jc : jc + 1]
                        )
                        pT.append(pT_sb)

                    for k in range(4):
                        i = 4 * gi + k
                        sl = slice(128 * k, 128 * (k + 1))
                        # hw[t] = h . w_h partials (plain f32: f32r needs N>=2);
                        # two single-shot columns, merged multiplicatively in exp
                        nc.tensor.matmul(
                            pq[:, 3 * NT + i : 3 * NT + i + 1], fview(hT[:, 0, sl]),
                            wh_col[:, 0:1], start=True, stop=True,
                        )
                        nc.tensor.matmul(
                            pq[:, 4 * NT + i : 4 * NT + i + 1],
                            fview(hT[0:72, 1, sl]),
                            wh_col[0:72, 1:2], start=True, stop=True,
                        )
                        # max_j exp(s0+uw): transpose p back to t-partitions, reduce
                        pm = pmc.tile([128, 256], DT, tag="mc", name="pm")
                        nc.tensor.transpose(pm[:, 0:128], pT[0][:, sl], ident)
                        nc.tensor.transpose(pm[:, 128:256], pT[1][:, sl], ident)
                        nc.vector.reduce_max(mcol[:, i : i + 1], pm, axis=AX.X)
                        # c2q (+ denominator via the ones column of u2)
                        pc = pmc.tile([128, 256], F32, tag="mc", name="pc")
                        nc.tensor.matmul(
                            pc, pT[0][:, sl], u2[:, 0, :], start=True, stop=False
                        )
                        nc.tensor.matmul(
                            pc, pT[1][:, sl], u2[:, 1, :], start=False, stop=True
                        )
                        rc = rcp.tile([128, 1], F32, tag="rc")
                        nc.vector.reciprocal(rc, pc[:, D : D + 1])
                        # alternate the normalize-copy between ACT and DVE to
                        # balance engine load
                        if i % 2 == 0:
                            nc.scalar.mul(g[:, i, D : 2 * D], pc[:, 0:D], mul=rc)
                        else:
                            nc.vector.tensor_scalar_mul(
                                g[:, i, D : 2 * D], pc[:, 0:D], rc
                            )
                        nc.vector.tensor_mul(
                            g[:, i, 2 * D : 3 * D], g[:, i, 0:D], g[:, i, D : 2 * D]
                        )

                    # T-softmax weights for this group: em = exp(m) =
                    # mcol * exp(hw0) * exp(hw1)  (mcol is exp-domain already)
                    sl4 = slice(4 * gi, 4 * gi + 4)
                    slh0 = slice(3 * NT + 4 * gi, 3 * NT + 4 * gi + 4)
                    slh1 = slice(4 * NT + 4 * gi, 4 * NT + 4 * gi + 4)
                    e1 = rcp.tile([128, 4], F32, tag="e1", name="e1")
                    nc.scalar.activation(ehw[:, sl4], pq[:, slh0], AF.Exp)
                    nc.scalar.activation(e1, pq[:, slh1], AF.Exp)
                    nc.vector.tensor_mul(em[:, sl4], mcol[:, sl4], ehw[:, sl4])
                    nc.vector.tensor_mul(em[:, sl4], em[:, sl4], e1)
                    for k in range(4):
                        i = 4 * gi + k
                        nc.tensor.matmul(
                            pq[:, i : i + 1], g[:, i, 0:128], em[:, i : i + 1],
                            start=True, stop=True,
                        )
                        nc.tensor.matmul(
                            pq[0:72, NT + i : NT + i + 1], g[:, i, 128:200],
                            em[:, i : i + 1], start=True, stop=True,
                        )

                    # stream this group's bulk output (cols 0:600) now -- the
                    # store ring fills while later groups still compute
                    nc.sync.dma_start(
                        out=g_d[b].rearrange("(n p) f -> p n f", p=128)[
                            :, 4 * gi : 4 * gi + 4, 0 : 3 * D
                        ],
                        in_=g[:, 4 * gi : 4 * gi + 4, 0 : 3 * D],
                    )

                # ---- batch tail: fold q2c columns, broadcast, final product ----
                nc.tensor.matmul(
                    pq[0:1, 2 * NT : 3 * NT], ones_col, em, start=True, stop=True
                )
                rts = smalls.tile([1, 1], F32, tag="rts")
                nc.vector.reduce_sum(rts, pq[0:1, 2 * NT : 3 * NT], axis=AX.X)
                nc.vector.reciprocal(rts, rts)
                q2cT = smalls.tile([128, 2], F32, tag="q2cT")
                nc.vector.reduce_sum(q2cT[:, 0:1], pq[:, 0:NT], axis=AX.X)
                nc.vector.reduce_sum(q2cT[0:72, 1:2], pq[0:72, NT : 2 * NT], axis=AX.X)
                pqr = pp.tile([1, 256], F32, tag="q", name="pqr")
                nc.tensor.transpose(pqr[0:1, 0:128], q2cT[:, 0:1], ident_f)
                nc.tensor.transpose(
                    pqr[0:1, 128:200], q2cT[0:72, 1:2], ident_f[0:72, 0:72]
                )
                qr_sb = smalls.tile([1, 256], F32, tag="qr")
                nc.vector.tensor_scalar_mul(qr_sb[0:1, 0:D], pqr[0:1, 0:D], rts)
                qb = smalls.tile([128, D], F32, tag="qb")
                nc.gpsimd.partition_broadcast(qb, qr_sb[0:1, 0:D])
                for i in range(NT):
                    nc.gpsimd.tensor_mul(g[:, i, 3 * D : 4 * D], g[:, i, 0:D], qb)

                nc.sync.dma_start(
                    out=g_d[b].rearrange("(n p) f -> p n f", p=128)[:, :, 3 * D :],
                    in_=g[:, :, 3 * D :],
                )
    nc.finalize()
    return nc


def kernel(**inputs):
    h = np.ascontiguousarray(np.asarray(inputs["h"], dtype=np.float32))
    u = np.ascontiguousarray(np.asarray(inputs["u"], dtype=np.float32))
    w_h = np.asarray(inputs["w_h"], dtype=np.float32)
    w_u = np.asarray(inputs["w_u"], dtype=np.float32)
    w_hu = np.asarray(inputs["w_hu"], dtype=np.float32)

    if "nc" not in _cache:
        _cache["nc"] = build_nc()
    nc = _cache["nc"]

    in_maps = []
    for c in range(N_CORES):
        sl = slice(c * BL, (c + 1) * BL)
        in_maps.append(
            {"h": h[sl], "u": u[sl], "w_h": w_h, "w_u": w_u, "w_hu": w_hu}
        )
    res = run_bass_kernel_spmd(nc, in_maps, list(range(N_CORES)))
    out = np.concatenate([res.results[c]["g"] for c in range(N_CORES)], axis=0)
    return out


# revision 50
# speedup vs baseline: 25.4531x; 25.4531x over previous
"""BiDAF attention-flow kernel for Trainium2 (8 NeuronCores, data-parallel over batch).

Problem shapes (hardcoded): h (64, 2048, 200), u (64, 256, 200), w_* (200,), b_* (1,).
Output g (64, 2048, 800) = concat([h, c2q, h*c2q, h*q2c], -1).

Sharding: batch 64 -> 8 cores x 8 batches. Weights replicated. No collectives.

Per-core per-batch math (T=2048, J=256, D=200):
  s[t,j]  = h.w_h[t] + u.w_u[j] + (h*w_hu).u[t,j]   (+ bias, a uniform shift of s that
            cancels in both softmaxes and is therefore dropped; b_* are zeros anyway)
  a       = softmax_j(s); c2q = a @ u
  m[t]    = max_j s ; beta = softmax_t(m); q2c = beta @ h

The kernel computes S^T = umod @ h^T with j on partitions (lhsT = umodT built once per
batch) so the exp'd scores are already j-partitioned for the c2q matmul -- no per-tile
transpose of the softmax output. h^T comes from PE transposes. The constant shift
invariance lets us skip max-subtraction inside both exps (|s| <= ~12 here, exp is safe
in fp32). uw (u.w_u) enters as the per-partition bias of the exp over S^T (partitions
are j there); hw (h.w_h) is computed as N=1 f32 matmuls from h^T and folded into the
T-softmax weights multiplicatively (em = rowmax_exp * exp(hw0) * exp(hw1)).

The big matmuls (S^T, c2q, uw) run in float32r: full-rate PE (1 cyc/row at N>=256) vs
4 cyc/row for fp32.  float32r STORAGE rounds values (~tf32), and walrus requires every
producer feeding an f32r matmul to emit f32r -- so the conversions happen in ACT
copies / exp (legal f32r producers), while the g output tile stays exact fp32
(h passes through bit-exact).  f32r matmuls also require N>=2, so the tiny N=1
matmuls (hw, q2c) run in plain fp32 via bitcast views of the f32r hT (bit-exact read).

Pipelining: inputs prefetch one batch ahead on the SWDGE (gpsimd) DMA ring so they
never queue behind output stores on the SP HWDGE ring; the u-side prep (umodT, uw)
is also computed one batch ahead; bulk output (cols 0:600) streams per 512-token
group; every per-tile accumulator matmul writes its own PSUM column (interleaved
accumulation groups sharing a bank clobber each other's has_written state).
Cost-model simulated time: ~176 us/core (HBM roofline for the 67 MB/core of traffic
is ~187 us on one ring).
"""

import sys

sys.path.insert(0, "/opt/trn_rl_repo")

import numpy as np

import concourse.bass as bass
import concourse.bacc as bacc
import concourse.mybir as mybir
import concourse.tile as tile
from concourse.bass_utils import run_bass_kernel_spmd

B, T, J, D = 64, 2048, 256, 200
N_CORES = 8
BL = B // N_CORES          # batches per core
NT = T // 128              # 16 t-tiles per batch
NG = NT // 4               # 4 groups of 512 tokens
F32 = mybir.dt.float32
F32R = mybir.dt.float32r
AX = mybir.AxisListType
AF = mybir.ActivationFunctionType

_cache = {}


def build_nc(use_f32r=True):
    DT = F32R if use_f32r else F32

    def dview(ap):  # DRAM-side view matching DT
        return ap.bitcast(F32R) if use_f32r else ap

    def fview(ap):  # f32 view of a DT-typed AP (bit-exact)
        return ap.bitcast(F32) if use_f32r else ap

    nc = bacc.Bacc()
    h_d = nc.dram_tensor("h", [BL, T, D], F32, kind="ExternalInput")
    u_d = nc.dram_tensor("u", [BL, J, D], F32, kind="ExternalInput")
    wh_d = nc.dram_tensor("w_h", [D], F32, kind="ExternalInput")
    wu_d = nc.dram_tensor("w_u", [D], F32, kind="ExternalInput")
    whu_d = nc.dram_tensor("w_hu", [D], F32, kind="ExternalInput")
    g_d = nc.dram_tensor("g", [BL, T, 4 * D], F32, kind="ExternalOutput")

    with tile.TileContext(nc) as tc:
        with (
            tc.tile_pool(name="sing", bufs=1) as sing,
            tc.tile_pool(name="gpool", bufs=3) as gpool,
            tc.tile_pool(name="upool", bufs=2) as upool,
            tc.tile_pool(name="lhsu", bufs=2) as lhsu,
            tc.tile_pool(name="htp", bufs=2) as htp,
            tc.tile_pool(name="ptp", bufs=2) as ptp,
            tc.tile_pool(name="smalls", bufs=2) as smalls,
            tc.tile_pool(name="rcp", bufs=3) as rcp,
            tc.tile_pool(name="pp", bufs=1, space="PSUM") as pp,
            tc.tile_pool(name="pmc", bufs=2, space="PSUM") as pmc,
            tc.tile_pool(name="pb", bufs=1, space="PSUM") as pb,
        ):
            # ---- per-core constants ----
            # (memset/affine_select can't emit f32r, and every producer feeding an
            # f32r matmul must be f32r-typed -- so constants come via inline DRAM.)
            ident_d = nc.inline_tensor(np.eye(128, dtype=np.float32), name="ident_d")
            ident = sing.tile([128, 128], DT)       # for f32r transposes (pT, uT)
            nc.sync.dma_start(out=ident, in_=dview(ident_d[:, :]))
            ident_f = sing.tile([128, 128], F32)    # for f32 transposes (h, q2c)
            nc.sync.dma_start(out=ident_f, in_=ident_d[:, :])
            ones_d = nc.inline_tensor(np.ones((128, 1), dtype=np.float32), name="ones_d")
            ones_col = sing.tile([128, 1], F32)
            nc.sync.dma_start(out=ones_col, in_=ones_d[:, :])
            # u2 pad columns 200:256 = [1, 0, 0, ...] (denominator ones column)
            upad_d = nc.inline_tensor(
                np.eye(1, 56, dtype=np.float32)[0], name="upad_d"
            )
            wh_col = sing.tile([128, 2], F32)   # hw matmuls run in plain f32 (N=1)
            wu_col = sing.tile([128, 2], DT)
            whu_col = sing.tile([128, 2], F32)  # tensor_scalar scalar must be f32
            wap = dview(wu_d.rearrange("(p o) -> p o", o=1))
            nc.sync.dma_start(out=wu_col[:, 0:1], in_=wap[0:128])
            nc.sync.dma_start(out=wu_col[0:72, 1:2], in_=wap[128:200])
            for w_dram, col in ((wh_d, wh_col), (whu_d, whu_col)):
                wap2 = w_dram.rearrange("(p o) -> p o", o=1)
                nc.sync.dma_start(out=col[:, 0:1], in_=wap2[0:128])
                nc.sync.dma_start(out=col[0:72, 1:2], in_=wap2[128:200])

            def load_batch(b):
                # loads go on the SWDGE (gpsimd) ring so they never queue behind
                # the output stores on the SP HWDGE ring
                g = gpool.tile([128, NT, 4 * D], F32, tag="g", name=f"g{b}")
                u2 = upool.tile([128, 2, 256], DT, tag="u", name=f"u2{b}")
                nc.gpsimd.dma_start(
                    out=g[:, :, 0:D], in_=h_d[b].rearrange("(n p) d -> p n d", p=128)
                )
                nc.gpsimd.dma_start(
                    out=u2[:, :, 0:D],
                    in_=dview(u_d[b].rearrange("(c p) d -> p c d", p=128)),
                )
                # cols 200:256: zero, with a ones column at 200 (softmax denominator)
                upad_ap = bass.AP(
                    tensor=upad_d[:].tensor,
                    offset=upad_d[:].offset,
                    ap=[[0, 128], [0, 2], [1, 56]],
                )
                nc.gpsimd.dma_start(out=u2[:, :, D : D + 56], in_=dview(upad_ap))
                return g, u2

            def prep_batch(b, g, u2):
                # ---- u-side prep: uT, uw row, fold w_hu ----
                lhsU0 = lhsu.tile([128, 256], DT, tag="l0", name=f"lhsU0_{b}")
                lhsU1 = lhsu.tile([128, 256], DT, tag="l1", name=f"lhsU1_{b}")
                uT0 = pmc.tile([128, 256], DT, tag="mc", name=f"uT0_{b}")
                for c in range(2):
                    nc.tensor.transpose(
                        uT0[:, 128 * c : 128 * (c + 1)], u2[:, c, 0:128], ident
                    )
                nc.scalar.copy(lhsU0, uT0)
                uT1 = pmc.tile([128, 256], DT, tag="mc", name=f"uT1_{b}")
                for c in range(2):
                    nc.tensor.transpose(
                        uT1[0:72, 128 * c : 128 * (c + 1)], u2[:, c, 128:200], ident
                    )
                nc.scalar.copy(lhsU1[0:72], uT1[0:72])
                # uw[j] = sum_d u[j,d] w_u[d]  (raw uT, before the w_hu fold)
                uwp = pb.tile([1, 272], F32, tag="b1", name=f"uwp_{b}")
                nc.tensor.matmul(
                    uwp[0:1, 0:256], wu_col[:, 0:1], lhsU0, start=True, stop=False
                )
                nc.tensor.matmul(
                    uwp[0:1, 0:256], wu_col[0:72, 1:2], lhsU1[0:72],
                    start=False, stop=True,
                )
                uw_sb = smalls.tile([1, 256], F32, tag="uw", name=f"uw_sb_{b}")
                nc.scalar.copy(uw_sb, uwp[0:1, 0:256])
                # uw as per-j columns: it becomes the bias of the exp over S^T
                # (partitions = j there), so it never enters the matmul at all.
                # Row -> column via two cross-partition SBUF->SBUF DMAs.
                uwcol = smalls.tile([128, 2], F32, tag="uwcol", name=f"uwcol_{b}")
                nc.gpsimd.dma_start(out=uwcol[:, 0:1], in_=uw_sb[0:1, 0:128])
                nc.gpsimd.dma_start(out=uwcol[:, 1:2], in_=uw_sb[0:1, 128:256])
                # fold w_hu into uT (in place)
                nc.vector.tensor_scalar_mul(lhsU0, lhsU0, whu_col[:, 0:1])
                nc.vector.tensor_scalar_mul(
                    lhsU1[0:72], lhsU1[0:72], whu_col[0:72, 1:2]
                )
                return lhsU0, lhsU1, uwcol

            g0, u20 = load_batch(0)
            pending = (g0, u20) + prep_batch(0, g0, u20)
            for b in range(BL):
                g, u2, lhsU0, lhsU1, uwcol = pending
                if b + 1 < BL:
                    gn, u2n = load_batch(b + 1)
                    pending = (gn, u2n) + prep_batch(b + 1, gn, u2n)

                mcol = smalls.tile([128, NT], F32, tag="mcol")
                em = smalls.tile([128, NT], F32, tag="em")
                ehw = smalls.tile([128, NT], F32, tag="ehw")
                # Per-tile matmul scratch: every N=1 matmul writes its OWN column
                # with start=stop=True, so there are no interleaved accumulation
                # groups sharing a bank (those clobber each other's has_written
                # state).  Columns: 0:16 q2c d<128, 16:32 q2c d>=128, 32:48
                # per-tile em sums, 48:64 hw d<128 partial, 64:80 hw d>=128
                # partial (merged later as exp(a+b)=exp(a)*exp(b)).
                pq = pp.tile([128, 80], F32, tag="q", name="pq")

                for gi in range(NG):
                    # h^T for this 512-token group (f32 transposes of the exact
                    # h block; the ACT psum->sbuf copy does the f32r rounding)
                    hT = htp.tile([128, 2, 512], DT, tag="hT")
                    phT0 = pp.tile([128, 512], F32, tag="hT0")
                    phT1 = pp.tile([128, 512], F32, tag="hT1")
                    for k in range(4):
                        i = 4 * gi + k
                        nc.tensor.transpose(
                            phT0[:, 128 * k : 128 * (k + 1)], g[:, i, 0:128], ident_f
                        )
                        nc.tensor.transpose(
                            phT1[0:72, 128 * k : 128 * (k + 1)], g[:, i, 128:200],
                            ident_f,
                        )
                    nc.scalar.copy(hT[:, 0, :], phT0)
                    nc.scalar.copy(hT[0:72, 1, :], phT1[0:72])

                    pT = []
                    for jc in range(2):
                        pST = pp.tile([128, 512], F32, tag=f"ST{jc}")
                        nc.tensor.matmul(
                            pST, lhsU0[:, 128 * jc : 128 * (jc + 1)], hT[:, 0, :],
                            start=True, stop=False,
                        )
                        nc.tensor.matmul(
                            pST, lhsU1[0:72, 128 * jc : 128 * (jc + 1)], hT[0:72, 1, :],
                            start=False, stop=True,
                        )
                        pT_sb = ptp.tile([128, 512], DT, tag=f"pT{jc}", name=f"pT{jc}")
                        # exp(s0 + uw[j]): uw rides as the per-partition bias
                        nc.scalar.activation(
                            pT_sb, pST, AF.Exp, bias=uwcol[:, jc : jc + 1]
                        )
                        pT.append(pT_sb)

                    for k in range(4):
                        i = 4 * gi + k
                        sl = slice(128 * k, 128 * (k + 1))
                        # hw[t] = h . w_h partials (plain f32: f32r needs N>=2);
                        # two single-shot columns, merged multiplicatively in exp
                        nc.tensor.matmul(
                            pq[:, 3 * NT + i : 3 * NT + i + 1], fview(hT[:, 0, sl]),
                            wh_col[:, 0:1], start=True, stop=True,
                        )
                        nc.tensor.matmul(
                            pq[:, 4 * NT + i : 4 * NT + i + 1],
                            fview(hT[0:72, 1, sl]),
                            wh_col[0:72, 1:2], start=True, stop=True,
                        )
                        # max_j exp(s0+uw): transpose p back to t-partitions, reduce
                        pm = pmc.tile([128, 256], DT, tag="mc", name="pm")
                        nc.tensor.transpose(pm[:, 0:128], pT[0][:, sl], ident)
                        nc.tensor.transpose(pm[:, 128:256], pT[1][:, sl], ident)
                        nc.vector.reduce_max(mcol[:, i : i + 1], pm, axis=AX.X)
                        # c2q (+ denominator via the ones column of u2)
                        pc = pmc.tile([128, 256], F32, tag="mc", name="pc")
                        nc.tensor.matmul(
                            pc, pT[0][:, sl], u2[:, 0, :], start=True, stop=False
                        )
                        nc.tensor.matmul(
                            pc, pT[1][:, sl], u2[:, 1, :], start=False, stop=True
                        )
                        rc = rcp.tile([128, 1], F32, tag="rc")
                        nc.vector.reciprocal(rc, pc[:, D : D + 1])
                        # alternate the normalize-copy between ACT and DVE to
                        # balance engine load
                        if i % 2 == 0:
                            nc.scalar.mul(g[:, i, D : 2 * D], pc[:, 0:D], mul=rc)
                        else:
                            nc.vector.tensor_scalar_mul(
                                g[:, i, D : 2 * D], pc[:, 0:D], rc
                            )
                        nc.vector.tensor_mul(
                            g[:, i, 2 * D : 3 * D], g[:, i, 0:D], g[:, i, D : 2 * D]
                        )

                    # T-softmax weights for this group: em = exp(m) =
                    # mcol * exp(hw0) * exp(hw1)  (mcol is exp-domain already)
                    sl4 = slice(4 * gi, 4 * gi + 4)
                    slh0 = slice(3 * NT + 4 * gi, 3 * NT + 4 * gi + 4)
                    slh1 = slice(4 * NT + 4 * gi, 4 * NT + 4 * gi + 4)
                    e1 = rcp.tile([128, 4], F32, tag="e1", name="e1")
                    nc.scalar.activation(ehw[:, sl4], pq[:, slh0], AF.Exp)
                    nc.scalar.activation(e1, pq[:, slh1], AF.Exp)
                    nc.vector.tensor_mul(em[:, sl4], mcol[:, sl4], ehw[:, sl4])
                    nc.vector.tensor_mul(em[:, sl4], em[:, sl4], e1)
                    for k in range(4):
                        i = 4 * gi + k
                        nc.tensor.matmul(
                            pq[:, i : i + 1], g[:, i, 0:128], em[:, i : i + 1],
                            start=True, stop=True,
                        )
                        nc.tensor.matmul(
                            pq[0:72, NT + i : NT + i + 1], g[:, i, 128:200],
                            em[:, i : i + 1], start=True, stop=True,
                        )

                    # stream this group's bulk output (cols 0:600) now -- the
                    # store ring fills while later groups still compute
                    nc.sync.dma_start(
                        out=g_d[b].rearrange("(n p) f -> p n f", p=128)[
                            :, 4 * gi : 4 * gi + 4, 0 : 3 * D
                        ],
                        in_=g[:, 4 * gi : 4 * gi + 4, 0 : 3 * D],
                    )

                # ---- batch tail: fold q2c columns, broadcast, final product ----
                nc.tensor.matmul(
                    pq[0:1, 2 * NT : 3 * NT], ones_col, em, start=True, stop=True
                )
                rts = smalls.tile([1, 1], F32, tag="rts")
                nc.vector.reduce_sum(rts, pq[0:1, 2 * NT : 3 * NT], axis=AX.X)
                nc.vector.reciprocal(rts, rts)
                q2cT = smalls.tile([128, 2], F32, tag="q2cT")
                nc.vector.reduce_sum(q2cT[:, 0:1], pq[:, 0:NT], axis=AX.X)
                nc.vector.reduce_sum(q2cT[0:72, 1:2], pq[0:72, NT : 2 * NT], axis=AX.X)
                pqr = pp.tile([1, 256], F32, tag="q", name="pqr")
                nc.tensor.transpose(pqr[0:1, 0:128], q2cT[:, 0:1], ident_f)
                nc.tensor.transpose(
                    pqr[0:1, 128:200], q2cT[0:72, 1:2], ident_f[0:72, 0:72]
                )
                qr_sb = smalls.tile([1, 256], F32, tag="qr")
                nc.vector.tensor_scalar_mul(qr_sb[0:1, 0:D], pqr[0:1, 0:D], rts)
                qb = smalls.tile([128, D], F32, tag="qb")
                nc.gpsimd.partition_broadcast(qb, qr_sb[0:1, 0:D])
                for i in range(NT):
                    nc.gpsimd.tensor_mul(g[:, i, 3 * D : 4 * D], g[:, i, 0:D], qb)
                # tail store on the SWDGE ring: keeps the SP ring free for the
                # next batch's bulk stores
                nc.gpsimd.dma_start(
                    out=g_d[b].rearrange("(n p) f -> p n f", p=128)[:, :, 3 * D :],
                    in_=g[:, :, 3 * D :],
                )
    nc.finalize()
    return nc


def _make_runner(nc):
    """jit-compiled SPMD runner (cached across kernel() calls; the library
    path in run_bass_kernel_spmd retraces on every invocation)."""
    import jax
    from jax.sharding import Mesh, PartitionSpec
    from jax.experimental.shard_map import shard_map
    from concourse import bass2jax
    from concourse.bass2jax import _bass_exec_p, install_neuronx_cc_hook

    install_neuronx_cc_hook()
    partition_name = nc.partition_id_tensor.name if nc.partition_id_tensor else None
    in_names, out_names, out_avals, zero_outs = [], [], [], []
    for alloc in nc.m.functions[0].allocations:
        if not isinstance(alloc, mybir.MemoryLocationSet):
            continue
        name = alloc.memorylocations[0].name
        if alloc.kind == "ExternalInput":
            if name != partition_name:
                in_names.append(name)
        elif alloc.kind == "ExternalOutput":
            out_names.append(name)
            shape = tuple(alloc.tensor_shape)
            dtype = mybir.dt.np(alloc.dtype)
            out_avals.append(jax.core.ShapedArray(shape, dtype))
            zero_outs.append(np.zeros(shape, dtype))
    all_in_names = in_names + out_names
    if partition_name is not None:
        all_in_names = all_in_names + [partition_name]

    def _body(*args):
        operands = list(args)
        if partition_name is not None:
            operands.append(bass2jax.partition_id_tensor())
        return tuple(
            _bass_exec_p.bind(
                *operands,
                out_avals=tuple(out_avals),
                in_names=tuple(all_in_names),
                out_names=tuple(out_names),
                lowering_input_output_aliases=(),
                sim_require_finite=True,
                sim_require_nnan=True,
                nc=nc,
            )
        )

    devices = jax.devices()[:N_CORES]
    mesh = Mesh(np.asarray(devices), ("core",))
    n_all = len(in_names) + len(out_names)
    sharded = jax.jit(
        shard_map(
            _body, mesh=mesh,
            in_specs=(PartitionSpec("core"),) * n_all,
            out_specs=(PartitionSpec("core"),) * len(out_names),
            check_rep=False,
        ),
        keep_unused=True,
    )
    zeros_cat = [np.zeros((N_CORES * z.shape[0], *z.shape[1:]), z.dtype)
                 for z in zero_outs]
    return sharded, in_names, zeros_cat


def kernel(**inputs):
    h = np.ascontiguousarray(np.asarray(inputs["h"], dtype=np.float32))
    u = np.ascontiguousarray(np.asarray(inputs["u"], dtype=np.float32))
    w_h = np.asarray(inputs["w_h"], dtype=np.float32)
    w_u = np.asarray(inputs["w_u"], dtype=np.float32)
    w_hu = np.asarray(inputs["w_hu"], dtype=np.float32)

    if "runner" not in _cache:
        _cache["nc"] = build_nc()
        _cache["runner"] = _make_runner(_cache["nc"])
    sharded, in_names, zeros_cat = _cache["runner"]

    full = {
        "h": h, "u": u,
        "w_h": np.concatenate([w_h] * N_CORES, axis=0),
        "w_u": np.concatenate([w_u] * N_CORES, axis=0),
        "w_hu": np.concatenate([w_hu] * N_CORES, axis=0),
    }
    args = [full[name] for name in in_names] + zeros_cat
    out = sharded(*args)
    g = np.asarray(out[0])          # (N_CORES*BL, T, 4D) == (B, T, 4D)
    return g.reshape(B, T, 4 * D)
